# revision 8
# baseline (speedup 1.0000x reference)
"""MoE block (top-2 of 8 experts) on 8 Trainium2 NeuronCores.

Strategy (expert-parallel, per sharding hint):
  - Host: router (logits = x @ Wg in fp64, top-2, renormalized gates),
    token dispatch: gather each expert's tokens, transpose to [D, C]
    feature-major layout, cast to bf16, pad to capacity CAP=2048
    (capacity factor 1.0). Tokens beyond an expert's capacity (162 at the
    graded seed) are computed exactly on the host in fp32 — a handful of
    rows through two BLAS matmuls — so accuracy is unaffected by capacity.
  - Device (SPMD, core e == expert e, no collectives): dense 2-layer FFN
    over the expert's token batch entirely in [feature, token] layout:
      hT = silu(W1.T-tiles @ xT)   [H, C]   (bf16 in, fp32 PSUM accum)
      yT = W2.T-tiles @ hT         [D, C]   (bf16 out; host combines fp32)
    Weights are used as matmul stationary operands in natural layout, so
    the kernel performs zero transposes.
  - Host: combine y = sum over the 2 selected experts of gate * y_e.

Kernel structure (v2, H-split single token block): the PE cost is
  streaming:  1024 weight tiles x CAP moving columns at 1 col/cycle
              (bf16) @2.4GHz  -> 874us at CAP=2048
  ldweights:  one serialized ~55ns InstLdweights per weight-tile visit
              (measured 54.7ns/ldweights on this HW by diffing builds
              with 2048 vs 5120 loads).
The old 2-block structure visited every weight tile twice (SBUF cannot
hold x plus a full-capacity h: 96B/token/partition), costing 2048 loads.
v2 instead splits H in half so h-half fits in SBUF at full capacity:
  P1: L1 over H[0:2048]    -> h0 [2048, C] bf16 in SBUF
  P2: L2 partial y0 = sum_{k2<16} W2[k2].T @ h0  -> bf16 DRAM scratch
  P3: L1 over H[2048:4096] -> h1 (reuses h0's SBUF)
  P4: L2 yT = y0 + sum_{k2>=16} W2[k2].T @ h1 (DVE add) -> bf16 out
Every weight tile is visited exactly once: 1024 ldweights (the floor).
The y0 roundtrip (bf16: 2x8.4MB) replaces the second weight streaming
pass of the 2-block structure (2x16.8MB), so total DMA drops ~17MB.
PSUM: [128,2048] fp32 accumulators are exactly 4 banks, double-buffered.

Post-legalization stream passes (validated bit-exact on HW by the
previous session): _strip_redundant_ldweights removes per-n-slice
reloads of the identical stationary tile; _thin_pe_sem_updates keeps
tile-clock sem-incs only on chain-ending matmuls.

Shapes are hardcoded for the graded problem:
  x [4, 2048, 2048] f32, Wg [2048, 8] f32, W1 [8, 2048, 4096] f32,
  W2 [8, 4096, 2048] f32, top_k = 2.
"""

import time

import numpy as np
import ml_dtypes

BF16 = ml_dtypes.bfloat16

B, S, D, H, E = 4, 2048, 2048, 4096, 8
T = B * S
NCORES = 8
CAP = 2048        # device per-expert token capacity (cf=1.0); any overflow
                  # is computed exactly on the host in fp32
C = CAP           # kept for test.py compatibility
KD = D // 128     # 16  L1 contraction tiles
MH2 = (H // 2) // 128   # 16  L1 output row-tiles per H half
KH2 = (H // 2) // 128   # 16  L2 contraction tiles per H half
MD = D // 128     # 16  L2 output row-tiles

_cache = {}


def _strip_redundant_ldweights(nc, mybir):
    """Remove InstLdweights that reload the exact weights already loaded
    into the PE array by the previous (surviving) InstLdweights — same
    tensor/offset/access-pattern/dtype, no semaphore waits/updates, and no
    intervening PE-state-changing instruction. Non-self-loading matmuls
    then reuse the loaded weights (bit-exact, validated on HW): saves the
    ~55ns serialized weight-load per matmul n-slice beyond the first."""
    PE = mybir.EngineType.PE
    removed = 0
    for func in nc.m.functions:
        for blk in func.blocks:
            insts = blk.instructions
            to_remove = []
            key = None
            for inst in insts:
                if getattr(inst, "engine", None) != PE:
                    continue
                nm = type(inst).__name__
                if nm == "InstLdweights":
                    si = inst.sync_info
                    has_sync = si is not None and (
                        len(si.on_wait) > 0 or len(si.on_update) > 0
                    )
                    if inst.is_transpose:
                        key = None
                        continue
                    ap = inst.ins[0].bass_ap
                    k = (
                        ap.tensor.name,
                        ap.offset,
                        str(ap.ap),
                        str(inst.ins[0].dtype),
                        str(inst.perf_mode),
                    )
                    if k == key and not has_sync:
                        to_remove.append(inst)
                        removed += 1
                    else:
                        key = k
                elif nm == "InstMatmult":
                    if inst.is_transpose:
                        key = None
                else:
                    key = None
            for inst in to_remove:
                insts.remove(inst)
    return removed


def _thin_pe_sem_updates(nc, mybir):
    """Keep tile-clock sem-incs only on stop_tensor_calc matmuls and
    renumber every wait on those sems (rounding up to the next kept inc,
    which only ever delays a waiter past more of the same accumulation
    chain — safe here since all cross-engine waits target chain ends or
    round up within the producing m-tile). Cuts EVT_SEM write traffic
    from one per matmul to one per accumulation chain."""
    for func in nc.m.functions:
        insts = [i for blk in func.blocks for i in blk.instructions]
        inc_seq = {}
        removed_any = False
        for inst in insts:
            si = inst.sync_info
            if si is None:
                continue
            is_mm = type(inst).__name__ == "InstMatmult"
            for up in si.on_update:
                if up.sync_type != "semaphore":
                    continue
                if (is_mm and up.update_mode == "sem-inc"
                        and up.update_value == 1):
                    keep = bool(inst.stop_tensor_calc)
                    inc_seq.setdefault(up.id, []).append(keep)
                    if not keep:
                        removed_any = True
                else:
                    inc_seq.setdefault(up.id, []).append(True)
        if not removed_any:
            continue
        kept_prefix = {}
        for sid, seq in inc_seq.items():
            pre = [0]
            for k in seq:
                pre.append(pre[-1] + (1 if k else 0))
            kept_prefix[sid] = (pre, seq)
        for inst in insts:
            si = inst.sync_info
            if si is None:
                continue
            new_waits = []
            changed = False
            for w in si.on_wait:
                if (w.sync_type == "semaphore" and w.id in kept_prefix
                        and w.wait_mode == "sem-ge-imm"
                        and w.wait_reg is None):
                    pre, seq = kept_prefix[w.id]
                    v = w.wait_value
                    if v is not None and 0 < v <= len(seq):
                        nv = pre[v]
                        if not seq[v - 1]:
                            nv += 1
                        if nv != v:
                            w = mybir.SyncWait(
                                sync_type=w.sync_type, id=w.id,
                                ant_name=w.ant_name, wait_mode=w.wait_mode,
                                wait_value=nv, wait_reg=w.wait_reg,
                            )
                            changed = True
                new_waits.append(w)
            if changed:
                inst.sync_info = mybir.SyncInfo(
                    on_wait=new_waits, on_update=list(si.on_update)
                )
        idx = {sid: 0 for sid in inc_seq}
        for inst in insts:
            si = inst.sync_info
            if si is None:
                continue
            is_mm = type(inst).__name__ == "InstMatmult"
            kept_updates = []
            changed = False
            for up in si.on_update:
                if up.sync_type == "semaphore" and up.id in inc_seq:
                    i = idx[up.id]
                    idx[up.id] += 1
                    if not inc_seq[up.id][i]:
                        changed = True
                        continue
                kept_updates.append(up)
            if changed:
                inst.sync_info = mybir.SyncInfo(
                    on_wait=list(inst.sync_info.on_wait),
                    on_update=kept_updates,
                )


def _build_bass_v2(optimize=True, cap=None):
    """H-split single-token-block kernel: 1024 ldweights (one per weight
    tile), CAP=2048 moving columns per tile. See module docstring."""
    import concourse.tile as tile
    from concourse import bacc, mybir
    from contextlib import ExitStack

    bf = mybir.dt.bfloat16
    f32 = mybir.dt.float32
    TB = CAP if cap is None else cap
    NSL = [(s, 512) for s in range(0, TB, 512)]

    nc = bacc.Bacc(
        "TRN2", target_bir_lowering=False, debug=False, num_devices=NCORES
    )
    xeT = nc.dram_tensor("xeT", [D, TB], bf, kind="ExternalInput").ap()
    w1 = nc.dram_tensor("w1", [D, H], bf, kind="ExternalInput").ap()
    w2 = nc.dram_tensor("w2", [H, D], bf, kind="ExternalInput").ap()
    yeT = nc.dram_tensor("yeT", [D, TB], bf, kind="ExternalOutput").ap()

    with tile.TileContext(nc) as tc, ExitStack() as ctx:
        # SBUF/partition budget (~198KB usable):
        #   x 16x4KB=64 | h 16x4KB=64 | w1 2x8KB=16 | w2 3x4KB=12
        #   y0out 2x4KB=8 | y0in 2x4KB=8 | out 2x8KB=16   -> 188KB
        xpool = ctx.enter_context(tc.tile_pool(name="xp", bufs=1))
        hpool = ctx.enter_context(tc.tile_pool(name="hp", bufs=1))
        w1pool = ctx.enter_context(tc.tile_pool(name="w1p", bufs=2))
        w2pool = ctx.enter_context(tc.tile_pool(name="w2p", bufs=3))
        y0opool = ctx.enter_context(tc.tile_pool(name="y0o", bufs=2))
        y0ipool = ctx.enter_context(tc.tile_pool(name="y0i", bufs=2))
        opool = ctx.enter_context(tc.tile_pool(name="op", bufs=2))
        pspool = ctx.enter_context(tc.tile_pool(name="ps", bufs=2, space="PSUM"))
        drampool = ctx.enter_context(tc.tile_pool(name="dram", bufs=1, space="DRAM"))

        # bf16 DRAM scratch for the first-half partial of y (dep-tracked
        # tile, so the P4 read-back orders after the P2 write)
        y0 = drampool.tile([128, MD, TB], bf, tag="y0")

        xts = [xpool.tile([128, TB], bf, tag=f"x{k}", name=f"x{k}") for k in range(KD)]

        def l1_phase(hh):
            """hT[m] = silu(sum_k W1[k, hh-half m].T @ xT[k]) for the 16
            m-tiles of one H half; returns the h tiles (bf16 [128, TB])."""
            hbase = hh * (H // 2)
            hts = []
            for mg in range(MH2 // 2):   # weight groups of 2 row-tiles
                w1all = w1pool.tile([128, KD * 256], bf, tag="w1g", name="w1all")
                if hh == 0 and mg == 0:
                    # cold start: per-k slice loads, x interleaved with
                    # weights, so the k=0 matmuls begin as soon as tile 0
                    # lands (region tracking gives per-slice deps)
                    for k in range(KD):
                        nc.sync.dma_start(
                            xts[k][:], xeT[k * 128:(k + 1) * 128, :]
                        )
                        nc.sync.dma_start(
                            w1all[:, k * 256:(k + 1) * 256],
                            w1[k * 128:(k + 1) * 128, hbase:hbase + 256],
                        )
                else:
                    # one 3D DMA for the whole group (fewer descriptors)
                    nc.sync.dma_start(
                        w1all[:].rearrange("p (k c) -> p k c", c=256),
                        w1.rearrange("(k p) h -> p k h", p=128)[
                            :, :, hbase + mg * 256:hbase + (mg + 1) * 256
                        ],
                    )
                for ml in range(2):
                    m = mg * 2 + ml
                    ps = pspool.tile([128, TB], f32, tag="ps", name="ps")
                    for k in range(KD):
                        lw = w1all[:, k * 256 + ml * 128:k * 256 + (ml + 1) * 128]
                        for (ns, nw) in NSL:
                            nc.tensor.matmul(
                                ps[:, ns:ns + nw],
                                lw,
                                xts[k][:, ns:ns + nw],
                                start=(k == 0),
                                stop=(k == KD - 1),
                            )
                    ht = hpool.tile([128, TB], bf, tag=f"h{m}", name=f"h{m}")
                    nc.scalar.activation(
                        ht[:], ps[:], mybir.ActivationFunctionType.Silu
                    )
                    hts.append(ht)
            return hts

        def l2_phase(hh, hts):
            """hh==0: y0[m2] = sum_{k2<16} W2[k2,m2].T @ h0[k2]  (bf16 scratch)
               hh==1: yeT[m2] = y0[m2] + sum_{k2} W2[16+k2,m2].T @ h1[k2]"""
            kbase = hh * KH2
            for m2 in range(MD):
                w2all = w2pool.tile([128, KH2 * 128], bf, tag="w2g", name="w2all")
                nc.sync.dma_start(
                    w2all[:].rearrange("p (k c) -> p k c", c=128),
                    w2.rearrange("(k p) d -> p k d", p=128)[
                        :, kbase:kbase + KH2, m2 * 128:(m2 + 1) * 128
                    ],
                )
                if hh == 1:
                    y0t = y0ipool.tile([128, TB], bf, tag="y0i", name="y0t")
                    nc.sync.dma_start(y0t[:], y0[:, m2, :])
                ps = pspool.tile([128, TB], f32, tag="ps", name="ps")
                for k2 in range(KH2):
                    lw = w2all[:, k2 * 128:(k2 + 1) * 128]
                    for (ns, nw) in NSL:
                        nc.tensor.matmul(
                            ps[:, ns:ns + nw],
                            lw,
                            hts[k2][:, ns:ns + nw],
                            start=(k2 == 0),
                            stop=(k2 == KH2 - 1),
                        )
                if hh == 0:
                    y0o = y0opool.tile([128, TB], bf, tag="y0o", name="y0o")
                    nc.vector.tensor_copy(y0o[:], ps[:])
                    nc.sync.dma_start(y0[:, m2, :], y0o[:])
                else:
                    ot = opool.tile([128, TB], bf, tag="o", name="ot")
                    nc.vector.tensor_tensor(
                        ot[:], ps[:], y0t[:], mybir.AluOpType.add
                    )
                    nc.sync.dma_start(yeT[m2 * 128:(m2 + 1) * 128, :], ot[:])

        h0 = l1_phase(0)
        l2_phase(0, h0)
        h1 = l1_phase(1)
        l2_phase(1, h1)

    if optimize:
        _strip_redundant_ldweights(nc, mybir)
        _thin_pe_sem_updates(nc, mybir)
    nc.compile()
    return nc


def _get_nc():
    """Build the optimized v2 kernel; on any failure of the post-scheduling
    stream passes fall back to a fresh un-optimized v2 build."""
    if "nc" not in _cache:
        try:
            _cache["nc"] = _build_bass_v2(optimize=True)
        except Exception:
            _cache["nc"] = _build_bass_v2(optimize=False)
    return _cache["nc"]


def _route(xt, Wg):
    """fp64 router: top-2 experts + renormalized gates per token."""
    logits = xt.astype(np.float64) @ Wg.astype(np.float64)        # [T, E]
    order = np.argsort(-logits, axis=1)
    top2 = order[:, :2]                                           # [T, 2]
    l2 = np.take_along_axis(logits, top2, axis=1)
    g = np.exp(l2 - l2.max(axis=1, keepdims=True))
    g = g / g.sum(axis=1, keepdims=True)                          # [T, 2]
    return top2, g


def kernel(x, Wg, W1, W2, top_k):
    from concourse.bass_utils import run_bass_kernel_spmd

    assert int(top_k) == 2
    x = np.asarray(x)
    Wg = np.asarray(Wg)
    W1 = np.asarray(W1)
    W2 = np.asarray(W2)
    xt = np.ascontiguousarray(x, dtype=np.float32).reshape(T, D)
    top2, gates = _route(xt, Wg)

    xT16 = np.ascontiguousarray(xt.T.astype(BF16))                # [D, T]

    idxs, slots = [], []
    for e in range(E):
        sel = np.where((top2 == e).any(axis=1))[0]
        idxs.append(sel)
        slots.append(np.argmax(top2[sel] == e, axis=1))

    in_maps = []
    for e in range(E):
        sel = idxs[e][:CAP]
        xeT = np.zeros((D, CAP), dtype=BF16)
        xeT[:, : len(sel)] = xT16[:, sel]
        in_maps.append(
            {
                "xeT": xeT,
                "w1": W1[e].astype(BF16),
                "w2": W2[e].astype(BF16),
            }
        )

    nc = _get_nc()
    try:
        res = run_bass_kernel_spmd(nc, in_maps, list(range(NCORES)))
    except Exception:
        # transient device/tunnel hiccups happen; one retry
        time.sleep(2)
        res = run_bass_kernel_spmd(nc, in_maps, list(range(NCORES)))

    out = np.zeros((T, D), dtype=np.float32)
    for e in range(E):
        sel = idxs[e][:CAP]
        ye = np.asarray(
            res.results[e]["yeT"][:, : len(sel)], dtype=np.float32
        )                                                         # [D, cnt]
        g = gates[idxs[e], slots[e]].astype(np.float32)
        out[sel] += g[: len(sel), None] * ye.T
        ov = idxs[e][CAP:]
        if len(ov):
            # capacity overflow: exact fp32 FFN on the host (few rows)
            z = xt[ov] @ np.asarray(W1[e], dtype=np.float32)
            hov = z / (1.0 + np.exp(-z))
            yov = hov @ np.asarray(W2[e], dtype=np.float32)
            out[ov] += g[len(sel):, None] * yov
    return out.reshape(B, S, D)
